# revision 12
# baseline (speedup 1.0000x reference)
"""Colight GNN message-passing kernel for 8x TRN2 NeuronCores (Bass/Tile).

Math (per head h of 5, per agent b, hidden dim d of 128, neighbors n of 8):
    actor = x @ Wa_h + ba_h                      [B, 128]
    other = others @ Wo_h + bo_h                 [8, B, 128]
    y     = mask ? actor * other : -inf
    e     = softmax_n(y)  (unnormalized: exp, sum, divide)
    fin   = others @ Wf_h + bf_h
    out_h = sum_n e * fin
    out   = relu((mean_h out_h) @ W_fc + b_fc)

Sharding: pure data-parallel over B across 8 cores (12500 agents/core).
Device layout: hidden dim d on the 128 SBUF partitions, agents b on the
free axis.  Host pre-transposes x/others to [64, b] so the contraction
dim k=64 sits on partitions for the PE.  Mask handled by adding
-1e30*(1-m) (broadcast along partitions via DMA) to the logits before
exp; softmax max-subtraction is skipped (|logits| <~ 15, exp is safe in
fp32 and exactly matches the reference softmax up to rounding).
"""

import os
import sys

import numpy as np

sys.path.insert(0, "/opt/trn_rl_repo")

import concourse.bass as bass
import concourse.tile as tile
from concourse import mybir
from concourse.vector_clock import ScopedClock


def _patched_drain_and_barrier(self, tick_clock, wait_clock):
    """Kernel-tail drain: this walrus build rejects an instruction carrying
    many sem waits ("Too many sync wait commands"), so spread the final
    clock-sync waits across single-wait nops before the drain."""
    nop_inst = self.nc.sync.nop(nofuse=True)
    wait_clock.add_sem_waits(
        nop_inst.ins, ScopedClock({None: tick_clock.global_clock})
    )
    waits = list(nop_inst.ins.sync_info.on_wait or [])
    if len(waits) > 1:
        nop_inst.ins.sync_info.on_wait = waits[:1]
        for w in waits[1:]:
            extra = self.nc.sync.nop(nofuse=True)
            if extra.ins.sync_info is None:
                extra.ins.sync_info = mybir.SyncInfo(on_wait=[], on_update=[])
            extra.ins.sync_info.on_wait = [w]
    self.nc.sync.drain()
    self.nc.all_engine_barrier()
    assert self.sems is not None
    popped = self.nc._tile_sem_poison_stack.pop()
    assert popped is self._sem_poison
    self.nc.clear_and_free_semaphores(list(self.sems.allocated().values()))
    self.nc.all_engine_barrier()


tile.TileContext._drain_and_barrier = _patched_drain_and_barrier

_WAIT_LIMIT = 1  # this walrus build rejects >1 sem wait per instruction


def _split_sync_waits(nc):
    """Move excess sem waits from any instruction onto preceding nofuse
    nops on the same engine (walrus 'Too many sync wait commands')."""
    n_split = 0
    for fn in nc.m.functions:
        for blk in fn.blocks:
            new_insts = []
            for inst in blk.instructions:
                si = inst.sync_info
                if si is not None and si.on_wait and len(si.on_wait) > _WAIT_LIMIT:
                    waits = list(si.on_wait)
                    for w in waits[:-_WAIT_LIMIT]:
                        n_split += 1
                        new_insts.append(
                            mybir.InstNoOp(
                                name=f"waitsplit_{n_split}_{inst.name}",
                                engine=inst.engine,
                                sync_info=mybir.SyncInfo(
                                    on_wait=[w], on_update=[]
                                ),
                                bass_nofuse=True,
                                text_hint="waitsplit",
                            )
                        )
                    si.on_wait = waits[-_WAIT_LIMIT:]
                new_insts.append(inst)
            blk.instructions = new_insts

B, N_NEI, IN_DIM, HEADS, HID = 100000, 8, 64, 5, 128
NCORES = 8
BSH = B // NCORES  # 12500 agents per core

AF = mybir.ActivationFunctionType
ALU = mybir.AluOpType
F32 = mybir.dt.float32
F32R = mybir.dt.float32r
BF16 = mybir.dt.bfloat16

MASK_NEG = -1.0e30


def build_nc(bshard: int, bc: int = 500, precise: bool = False, split_waits: bool = True):
    """Build the per-core Bass program.

    precise=False: e/fin/products/trees in bf16, mask applied post-exp as
    a 0/1 multiply, fin evacuated from PSUM by ScalarE with fused bias.
    precise=True: all-fp32 arithmetic (mask via -1e30 logit add).
    """
    assert bshard % bc == 0
    nchunks = bshard // bc
    nc = bass.Bass("TRN2", target_bir_lowering=False, debug=False)

    xT = nc.dram_tensor("xT", [IN_DIM, bshard], F32R, kind="ExternalInput").ap()
    othersT = nc.dram_tensor(
        "othersT", [N_NEI, IN_DIM, bshard], F32R, kind="ExternalInput"
    ).ap()
    # mk: precise -> -1e30*(1-m) logit bias; fast -> 0/1 multiplicative mask
    mk = nc.dram_tensor("mk", [N_NEI, bshard], BF16, kind="ExternalInput").ap()
    w3 = nc.dram_tensor("w3", [3, HEADS, IN_DIM, HID], F32R, kind="ExternalInput").ap()
    ball = nc.dram_tensor("ball", [HID, 3, HEADS], F32, kind="ExternalInput").ap()
    wfc = nc.dram_tensor("wfc", [HID, HID], F32, kind="ExternalInput").ap()
    bfc = nc.dram_tensor("bfc", [HID, 1], F32, kind="ExternalInput").ap()
    out = nc.dram_tensor("out", [HID, bshard], F32, kind="ExternalOutput").ap()

    FG = 2  # fin matmuls per PSUM group (2 banks)

    with tile.TileContext(nc) as tc:
        with (
            tc.tile_pool(name="singles", bufs=1) as singles,
            tc.tile_pool(name="io", bufs=2) as io,
            tc.tile_pool(name="work", bufs=2) as work,
            tc.tile_pool(name="small", bufs=2) as small,
            tc.tile_pool(name="ps_oth", bufs=2, space="PSUM") as ps_oth,
            tc.tile_pool(name="ps_fin", bufs=2, space="PSUM") as ps_fin,
            tc.tile_pool(name="ps_af", bufs=1, space="PSUM") as ps_af,
        ):
            w_sb = singles.tile([IN_DIM, 3, HEADS, HID], F32R)
            nc.sync.dma_start(out=w_sb, in_=w3.rearrange("t h k d -> k t h d"))
            ball_sb = singles.tile([HID, 3 * HEADS], F32)
            nc.sync.dma_start(out=ball_sb, in_=ball.rearrange("p t h -> p (t h)"))
            wfc_sb = singles.tile([HID, HID], F32)
            nc.sync.dma_start(out=wfc_sb, in_=wfc)
            bfc_sb = singles.tile([HID, 1], F32)
            nc.sync.dma_start(out=bfc_sb, in_=bfc)

            for c in range(nchunks):
                b0 = c * bc
                xT_c = io.tile([IN_DIM, bc], F32R)
                nc.sync.dma_start(out=xT_c, in_=xT[:, b0 : b0 + bc])
                oth_c = io.tile([IN_DIM, N_NEI, bc], F32R)
                nc.sync.dma_start(
                    out=oth_c,
                    in_=othersT[:, :, b0 : b0 + bc].rearrange("n k b -> k n b"),
                )
                mk_bc = io.tile([HID, N_NEI, bc], BF16)
                msl = mk[:, b0 : b0 + bc]
                nc.sync.dma_start(
                    out=mk_bc,
                    in_=bass.AP(
                        tensor=msl.tensor,
                        offset=msl.offset,
                        ap=[[0, HID]] + list(msl.ap),
                    ),
                )

                final_acc = small.tile([HID, bc], F32, tag="facc")

                for h in range(HEADS):
                    actor_ps = ps_af.tile([HID, bc], F32, tag="actor_ps")
                    nc.tensor.matmul(
                        out=actor_ps, lhsT=w_sb[:, 0, h, :], rhs=xT_c,
                        start=True, stop=True,
                    )
                    actor_sb = small.tile([HID, bc], F32, tag="actor_sb")
                    nc.scalar.activation(
                        out=actor_sb, in_=actor_ps, func=AF.Identity,
                        bias=ball_sb[:, 0 * HEADS + h : 0 * HEADS + h + 1],
                    )

                    # logits: y[n] = (oth_n + bo)*actor
                    y_buf = work.tile([HID, N_NEI, bc], F32, tag="y")
                    for n in range(N_NEI):
                        oth_ps = ps_oth.tile([HID, bc], F32, tag="oth_ps")
                        nc.tensor.matmul(
                            out=oth_ps, lhsT=w_sb[:, 1, h, :], rhs=oth_c[:, n, :],
                            start=True, stop=True,
                        )
                        nc.vector.scalar_tensor_tensor(
                            out=y_buf[:, n, :],
                            in0=oth_ps,
                            scalar=ball_sb[:, 1 * HEADS + h : 1 * HEADS + h + 1],
                            in1=actor_sb,
                            op0=ALU.add,
                            op1=ALU.mult,
                        )

                    if precise:
                        nc.vector.tensor_add(y_buf, y_buf, mk_bc)
                        e_sb = work.tile([HID, N_NEI, bc], F32, tag="e32")
                        nc.scalar.activation(out=e_sb, in_=y_buf, func=AF.Exp)
                        s_sb = small.tile([HID, bc], F32, tag="s")
                        nc.vector.tensor_reduce(
                            out=s_sb,
                            in_=e_sb.rearrange("p n b -> p b n"),
                            axis=mybir.AxisListType.X,
                            op=ALU.add,
                        )
                        p_buf = work.tile([HID, N_NEI, bc], F32, tag="y")
                        for g in range(N_NEI // FG):
                            fin_ps = ps_fin.tile([HID, FG, 512], F32, tag="fin_ps")
                            for j in range(FG):
                                nc.tensor.matmul(
                                    out=fin_ps[:, j, :bc],
                                    lhsT=w_sb[:, 2, h, :],
                                    rhs=oth_c[:, g * FG + j, :],
                                    start=True, stop=True,
                                )
                            nc.vector.scalar_tensor_tensor(
                                out=p_buf[:, g * FG : (g + 1) * FG, :],
                                in0=fin_ps[:, :, :bc],
                                scalar=ball_sb[:, 2 * HEADS + h : 2 * HEADS + h + 1],
                                in1=e_sb[:, g * FG : (g + 1) * FG, :],
                                op0=ALU.add,
                                op1=ALU.mult,
                            )
                        num_sb = small.tile([HID, bc], F32, tag="num")
                        nc.vector.tensor_reduce(
                            out=num_sb,
                            in_=p_buf.rearrange("p n b -> p b n"),
                            axis=mybir.AxisListType.X,
                            op=ALU.add,
                        )
                    else:
                        e_sb = work.tile([HID, N_NEI, bc], BF16, tag="e")
                        nc.scalar.activation(out=e_sb, in_=y_buf, func=AF.Exp)
                        eh_sb = work.tile([HID, N_NEI, bc], BF16, tag="eh")
                        nc.vector.tensor_mul(eh_sb, e_sb, mk_bc)

                        fin_sb = work.tile([HID, N_NEI, bc], BF16, tag="fin")
                        for g in range(N_NEI // FG):
                            fin_ps = ps_fin.tile([HID, FG, 512], F32, tag="fin_ps")
                            for j in range(FG):
                                nc.tensor.matmul(
                                    out=fin_ps[:, j, :bc],
                                    lhsT=w_sb[:, 2, h, :],
                                    rhs=oth_c[:, g * FG + j, :],
                                    start=True, stop=True,
                                )
                            nc.scalar.activation(
                                out=fin_sb[:, g * FG : (g + 1) * FG, :],
                                in_=fin_ps[:, :, :bc],
                                func=AF.Identity,
                                bias=ball_sb[:, 2 * HEADS + h : 2 * HEADS + h + 1],
                            )
                        p_sb = work.tile([HID, N_NEI, bc], BF16, tag="e")
                        nc.vector.tensor_mul(p_sb, eh_sb, fin_sb)

                        s_sb = small.tile([HID, bc], F32, tag="s")
                        num_sb = small.tile([HID, bc], F32, tag="num")
                        for src, dst in ((eh_sb, s_sb), (p_sb, num_sb)):
                            t1 = work.tile([HID, 4, bc], BF16, tag="t1")
                            nc.vector.tensor_add(t1, src[:, 0:4, :], src[:, 4:8, :])
                            t2 = work.tile([HID, 2, bc], BF16, tag="t2")
                            nc.vector.tensor_add(t2, t1[:, 0:2, :], t1[:, 2:4, :])
                            nc.vector.tensor_add(dst, t2[:, 0, :], t2[:, 1, :])

                    r_sb = small.tile([HID, bc], F32, tag="r")
                    nc.vector.reciprocal(out=r_sb, in_=s_sb)
                    if h == 0:
                        nc.vector.tensor_mul(final_acc, num_sb, r_sb)
                    else:
                        nr_sb = small.tile([HID, bc], F32, tag="nr")
                        nc.vector.tensor_mul(nr_sb, num_sb, r_sb)
                        nc.vector.tensor_add(final_acc, final_acc, nr_sb)

                fc_ps = ps_af.tile([HID, bc], F32, tag="fc_ps")
                nc.tensor.matmul(
                    out=fc_ps, lhsT=wfc_sb, rhs=final_acc, start=True, stop=True
                )
                out_sb = io.tile([HID, bc], F32, tag="out_sb")
                nc.scalar.activation(out=out_sb, in_=fc_ps, func=AF.Relu, bias=bfc_sb)
                nc.sync.dma_start(out=out[:, b0 : b0 + bc], in_=out_sb)

    if split_waits:
        _split_sync_waits(nc)
    return nc


def prep_host_inputs(
    x, others, neighbor_masks, Wa, ba, Wo, bo, Wf, bf, W_fc, b_fc,
    precise: bool = False,
):
    """Shared (weights) + per-core (shard) host-side prep -> in_maps."""
    import ml_dtypes

    bf16 = np.dtype(ml_dtypes.bfloat16)
    w3 = np.ascontiguousarray(np.stack([Wa, Wo, Wf], axis=0), dtype=np.float32)
    ball = np.ascontiguousarray(
        np.stack([ba, bo, bf], axis=0).transpose(2, 0, 1), dtype=np.float32
    )  # [128, 3, H]
    wfc = np.ascontiguousarray(W_fc, dtype=np.float32) / np.float32(HEADS)
    bfc = np.ascontiguousarray(b_fc, dtype=np.float32).reshape(HID, 1)

    mf = neighbor_masks.astype(np.float32)
    if precise:
        mk_full = (MASK_NEG * (1.0 - mf)).astype(bf16)  # [8, B]
    else:
        mk_full = mf.astype(bf16)

    in_maps = []
    for c in range(NCORES):
        s = slice(c * BSH, (c + 1) * BSH)
        in_maps.append(
            {
                "xT": np.ascontiguousarray(x[s].T, dtype=np.float32),
                "othersT": np.ascontiguousarray(
                    others[:, s, :].transpose(0, 2, 1), dtype=np.float32
                ),
                "mk": np.ascontiguousarray(mk_full[:, s]),
                "w3": w3,
                "ball": ball,
                "wfc": wfc,
                "bfc": bfc,
            }
        )
    return in_maps


def _ml_bf16():
    import ml_dtypes

    return np.dtype(ml_dtypes.bfloat16)


_CACHED_NC = None


PRECISE = False


def _get_nc():
    global _CACHED_NC
    if _CACHED_NC is None:
        _CACHED_NC = build_nc(BSH, precise=PRECISE)
    return _CACHED_NC


def kernel(**inputs):
    from concourse.bass_utils import run_bass_kernel_spmd

    in_maps = prep_host_inputs(**inputs, precise=PRECISE)
    res = run_bass_kernel_spmd(_get_nc(), in_maps, core_ids=list(range(NCORES)))
    outs = [res.results[c]["out"] for c in range(NCORES)]  # each [128, BSH]
    full = np.concatenate([o.T for o in outs], axis=0).astype(np.float32)
    return full


def profile_exec_ns(inputs):
    """Run with NTFF tracing and return profiled kernel exec time in ns."""
    from concourse.bass_utils import run_bass_kernel_spmd

    in_maps = prep_host_inputs(**inputs, precise=PRECISE)
    res = run_bass_kernel_spmd(
        _get_nc(), in_maps, core_ids=list(range(NCORES)), trace=True
    )
    return res.exec_time_ns
